# revision 4
# baseline (speedup 1.0000x reference)
"""Trainium2 Bass kernel for nn_HGNN_ATT (HGNN message passing, K sub-graphs).

Sharding: nodes re-permuted so every shard holds an equal mix of users and
items (shard m = users [6250m..) ++ items [6250m..)), padded to 12544 rows.
Each directed edge is owned by the core owning its accumulation target.
Per step: AllGather h -> dma_gather h[dst] chunks -> DVE scale by val ->
one-hot S per 128-edge tile (is_equal vs iota) -> PE matmul accumulated in
PSUM per 128-target window -> SBUF accumulator; repeat for second spmm over
the edge table; softmax/fc1/fusion-gate in feature-major layout.
"""

import numpy as np

import concourse.bass as bass
import concourse.mybir as mybir
import concourse.tile as tile
from concourse import bacc
from concourse.masks import make_identity

F32 = mybir.dt.float32
I16 = mybir.dt.int16
I32 = mybir.dt.int32
AF = mybir.ActivationFunctionType

NCORES = 8
D = 64
P = 128
BUCKET = 32768          # int16 gather index range
B_S = 4                 # tiles per S-build DVE instruction
CCH = 512               # post-phase column chunk (PSUM free-dim limit)


class Cfg:
    def __init__(self, NU, NI, K, E, CH):
        assert NU % NCORES == 0 and NI % NCORES == 0
        self.NU, self.NI, self.K, self.E, self.CH = NU, NI, K, E, CH
        self.UPC = NU // NCORES
        self.IPC = NI // NCORES
        self.SH = self.UPC + self.IPC            # real rows per shard
        self.SHP = ((self.SH + P - 1) // P) * P  # padded rows per shard
        self.NT = self.SHP // P                  # target windows per shard
        self.GN = NCORES * self.SHP              # padded global table rows
        self.NB = (self.GN + BUCKET - 1) // BUCKET


def _perm_maps(cfg):
    """original node id -> (core, local_row, padded_global_row)."""
    orig = np.arange(cfg.NU + cfg.NI)
    is_item = orig >= cfg.NU
    core = np.where(is_item, (orig - cfg.NU) // cfg.IPC, orig // cfg.UPC)
    loc = np.where(is_item, cfg.UPC + (orig - cfg.NU) % cfg.IPC, orig % cfg.UPC)
    g = core * cfg.SHP + loc
    return core.astype(np.int64), loc.astype(np.int64), g.astype(np.int64)


def prep(cfg, rows, cols):
    """Host-side graph preprocessing.  Returns (plan, per-core arrays)."""
    NU, NI, K, CH = cfg.NU, cfg.NI, cfg.K, cfg.CH
    N = NU + NI
    core_of, loc_of, g_of = _perm_maps(cfg)

    plan = {"nt": [], "chunks": []}
    gidx_cols = [[] for _ in range(NCORES)]
    val_cols = [[] for _ in range(NCORES)]
    srel_cols = [[] for _ in range(NCORES)]

    for k in range(K):
        r = np.asarray(rows[k]).astype(np.int64)
        c = np.asarray(cols[k]).astype(np.int64)
        src = np.concatenate([r, c + NU])
        dst = np.concatenate([c + NU, r])
        deg = np.bincount(src, minlength=N).astype(np.float32) + np.float32(1e-7)
        dinv = deg ** np.float32(-0.5)
        val = (dinv[src] * dinv[dst]).astype(np.float32)

        e_core = core_of[src]
        e_tloc = loc_of[src]                  # accumulation target (local row)
        e_grow = g_of[dst]                    # gather row (padded global)
        e_b = e_grow // BUCKET
        e_w = e_tloc // P

        cell_cnt = np.zeros((NCORES, cfg.NB, cfg.NT), np.int64)
        flat = (e_core * cfg.NB + e_b) * cfg.NT + e_w
        np.add.at(cell_cnt.reshape(-1), flat, 1)
        nt_k = np.ceil(cell_cnt.max(axis=0) / P).astype(np.int64)  # [NB, NT]
        plan["nt"].append(nt_k)

        chunks_k = []
        for b in range(cfg.NB):
            tot = int(nt_k[b].sum()) * P
            ch_list = []
            while tot > 0:
                L = min(CH, tot)
                ch_list.append(L)
                tot -= L
            chunks_k.append(ch_list)
        plan["chunks"].append(chunks_k)

        order = np.lexsort((e_grow, e_w, e_b, e_core))
        s_core = e_core[order]
        s_tloc = e_tloc[order]
        s_grow = e_grow[order]
        s_val = val[order]
        s_b = e_b[order]
        s_w = e_w[order]
        start = np.searchsorted(s_core, np.arange(NCORES))
        stop = np.searchsorted(s_core, np.arange(NCORES) + 1)

        for m in range(NCORES):
            mb = s_b[start[m]:stop[m]]
            mw = s_w[start[m]:stop[m]]
            mg = s_grow[start[m]:stop[m]]
            mt = s_tloc[start[m]:stop[m]]
            mv = s_val[start[m]:stop[m]]
            key = mb * cfg.NT + mw
            cs = np.searchsorted(key, np.arange(cfg.NB * cfg.NT))
            ce = np.searchsorted(key, np.arange(cfg.NB * cfg.NT) + 1)
            gi_parts, vl_parts, sr_parts = [], [], []
            for b in range(cfg.NB):
                for w in range(cfg.NT):
                    npad = int(nt_k[b, w]) * P
                    if npad == 0:
                        continue
                    a, z = cs[b * cfg.NT + w], ce[b * cfg.NT + w]
                    n = z - a
                    gi = np.full(npad, 0, np.int64)
                    vl = np.zeros(npad, np.float32)
                    sr = np.zeros(npad, np.float32)
                    gi[:n] = mg[a:z] - b * BUCKET
                    vl[:n] = mv[a:z]
                    sr[:n] = (mt[a:z] - w * P).astype(np.float32)
                    gi_parts.append(gi)
                    vl_parts.append(vl)
                    sr_parts.append(sr)
            gi_all = np.concatenate(gi_parts)
            vl_all = np.concatenate(vl_parts)
            sr_all = np.concatenate(sr_parts)
            off = 0
            for b in range(cfg.NB):
                for L in chunks_k[b]:
                    seg_g = gi_all[off:off + L].astype(np.int16)
                    gidx_cols[m].append(seg_g.reshape(L // 16, 16).T)
                    val_cols[m].append(vl_all[off:off + L].reshape(L // P, P).T)
                    srel_cols[m].append(sr_all[off:off + L].reshape(L // P, P).T)
                    off += L
            assert off == len(gi_all)

    per_core = []
    for m in range(NCORES):
        per_core.append({
            "gidx": np.ascontiguousarray(np.concatenate(gidx_cols[m], axis=1)),
            "val": np.ascontiguousarray(np.concatenate(val_cols[m], axis=1)),
            "srel": np.ascontiguousarray(np.concatenate(srel_cols[m], axis=1)),
        })
    return plan, per_core


def build(cfg, plan):
    nc = bacc.Bacc("TRN2", target_bir_lowering=False, debug=False,
                   num_devices=NCORES, num_swdge_queues=4,
                   dynamic_dma_scratch_size=131072)
    K, SHP, GN, NT, CH = cfg.K, cfg.SHP, cfg.GN, cfg.NT, cfg.CH
    TOTCOL = sum(L for k in range(K) for b in range(cfg.NB)
                 for L in plan["chunks"][k][b])

    xT_in = nc.declare_dram_parameter("xT", [D, SHP], F32, isOutput=False)
    biascol = nc.declare_dram_parameter("biascol", [D, 1], F32, isOutput=False)
    fc1_WT = nc.declare_dram_parameter("fc1_WT", [D, D], F32, isOutput=False)
    fus1_WT = nc.declare_dram_parameter("fus1_WT", [D, D], F32, isOutput=False)
    b1col = nc.declare_dram_parameter("b1col", [D, 1], F32, isOutput=False)
    w2col = nc.declare_dram_parameter("w2col", [D, 1], F32, isOutput=False)
    gidx_d = nc.declare_dram_parameter("gidx", [16, TOTCOL // 16], I16,
                                       isOutput=False)
    val_d = nc.declare_dram_parameter("val", [P, TOTCOL // P], F32,
                                      isOutput=False)
    srel_d = nc.declare_dram_parameter("srel", [P, TOTCOL // P], F32,
                                       isOutput=False)

    nodesT_o = nc.declare_dram_parameter("nodesT", [K, D, SHP], F32,
                                         isOutput=True)
    edges_o = nc.declare_dram_parameter("edges", [K, SHP, D], F32,
                                        isOutput=True)

    h_bounce = nc.dram_tensor("h_bounce", [SHP, D], F32)
    e_bounce = nc.dram_tensor("e_bounce", [SHP, D], F32)
    h_full = nc.dram_tensor("h_full", [GN, D], F32, addr_space="Shared")
    e_full = nc.dram_tensor("e_full", [GN, D], F32, addr_space="Shared")
    xT_d = nc.dram_tensor("xT_d", [D, SHP], F32)

    RG = [list(range(NCORES))]
    ccols = []
    o = 0
    while o < SHP:
        ccols.append((o, min(CCH, SHP - o)))
        o += min(CCH, SHP - o)

    with tile.TileContext(nc) as tc:
        with tc.tile_pool(name="persist", bufs=1) as pp:
            ident = pp.tile([P, P], F32)
            make_identity(nc, ident[:])
            iota_i = pp.tile([P, P], I32)
            nc.gpsimd.iota(iota_i[:], pattern=[[1, P]], base=0,
                           channel_multiplier=0)
            iota_f = pp.tile([P, P], F32)
            nc.vector.tensor_copy(iota_f[:], iota_i[:])
            acc = pp.tile([P, NT * D], F32)
            wfc1 = pp.tile([D, D], F32)
            nc.sync.dma_start(wfc1[:], fc1_WT[:, :])
            wfus = pp.tile([D, D], F32)
            nc.sync.dma_start(wfus[:], fus1_WT[:, :])
            bcol = pp.tile([D, 1], F32)
            nc.sync.dma_start(bcol[:], biascol[:, :])
            b1c = pp.tile([D, 1], F32)
            nc.sync.dma_start(b1c[:], b1col[:, :])
            w2c = pp.tile([D, 1], F32)
            nc.sync.dma_start(w2c[:], w2col[:, :])
            ones1 = pp.tile([1, D], F32)
            nc.vector.memset(ones1[:], 1.0)

            nc.sync.dma_start(xT_d[:, :], xT_in[:, :])

            qctr = [0]

            def spmm(k, ph, table, col_off):
                nt_k = plan["nt"][k]
                with tc.tile_pool(name=f"sp{k}{ph}", bufs=3) as sp, \
                     tc.tile_pool(name=f"spS{k}{ph}", bufs=6) as spS, \
                     tc.tile_pool(name=f"spP{k}{ph}", bufs=2,
                                  space="PSUM") as spP:
                    nc.vector.memset(acc[:], 0.0)
                    for b in range(cfg.NB):
                        base = b * BUCKET
                        nrow = min(BUCKET, GN - base)
                        tile_in_cell = 0
                        cell_iter = [(w, int(nt_k[b, w])) for w in range(NT)
                                     if nt_k[b, w] > 0]
                        ci = 0
                        cur_psum = None
                        for L in plan["chunks"][k][b]:
                            gi = sp.tile([P, CH // 16], I16, tag="gi")
                            nc.sync.dma_start(
                                gi[0:16, 0:L // 16],
                                gidx_d[:, col_off // 16:(col_off + L) // 16])
                            for lo, hi in ((16, 32), (32, 64), (64, 128)):
                                nc.sync.dma_start(
                                    gi[lo:hi, 0:L // 16],
                                    gi[0:lo, 0:L // 16][0:hi - lo, :])
                            vv = sp.tile([P, CH // P], F32, tag="vv")
                            nc.sync.dma_start(
                                vv[:, 0:L // P],
                                val_d[:, col_off // P:(col_off + L) // P])
                            sr = sp.tile([P, CH // P], F32, tag="sr")
                            nc.sync.dma_start(
                                sr[:, 0:L // P],
                                srel_d[:, col_off // P:(col_off + L) // P])
                            G = sp.tile([P, CH // P, D], F32, tag="G")
                            nc.gpsimd.dma_gather(
                                out_ap=G[:, 0:L // P, :],
                                in_ap=table[base:base + nrow, :],
                                idxs_ap=gi[:, 0:L // 16],
                                num_idxs=L, num_idxs_reg=L, elem_size=D,
                                single_packet=False, queue_num=qctr[0] % 4)
                            qctr[0] += 1
                            nc.vector.tensor_tensor(
                                out=G[:, 0:L // P, :], in0=G[:, 0:L // P, :],
                                in1=vv[:, 0:L // P].unsqueeze(2).to_broadcast(
                                    [P, L // P, D]),
                                op=mybir.AluOpType.mult)
                            ntile = L // P
                            S_tiles = []
                            for t0 in range(0, ntile, B_S):
                                bs = min(B_S, ntile - t0)
                                S = spS.tile([P, B_S, P], F32, tag="S")
                                nc.vector.tensor_tensor(
                                    out=S[:, 0:bs, :],
                                    in0=sr[:, t0:t0 + bs].unsqueeze(2)
                                        .to_broadcast([P, bs, P]),
                                    in1=iota_f[:].unsqueeze(1)
                                        .to_broadcast([P, bs, P]),
                                    op=mybir.AluOpType.is_equal)
                                S_tiles.append(S)
                            for t in range(ntile):
                                w, ntl = cell_iter[ci]
                                if tile_in_cell == 0:
                                    cur_psum = spP.tile([P, D], F32, tag="ps")
                                S = S_tiles[t // B_S]
                                last = tile_in_cell == ntl - 1
                                nc.tensor.matmul(
                                    out=cur_psum[:, :],
                                    lhsT=S[:, t % B_S, :],
                                    rhs=G[:, t, :],
                                    start=(tile_in_cell == 0), stop=last)
                                if last:
                                    nc.vector.tensor_tensor(
                                        out=acc[:, w * D:(w + 1) * D],
                                        in0=acc[:, w * D:(w + 1) * D],
                                        in1=cur_psum[:, :],
                                        op=mybir.AluOpType.add)
                                    tile_in_cell = 0
                                    ci += 1
                                else:
                                    tile_in_cell += 1
                            col_off += L
                        assert ci == len(cell_iter) and tile_in_cell == 0
                return col_off

            def h_chunk(sb, ps, xnew_s, co, cn):
                h_s = sb.tile([D, CCH], F32, tag="hs")
                nc.scalar.activation(h_s[:, 0:cn], xnew_s[:, 0:cn], AF.Relu)
                nc.vector.tensor_scalar_add(h_s[:, 0:cn], h_s[:, 0:cn],
                                            bcol[:, 0:1])
                for j in range(0, cn, P):
                    pj = min(P, cn - j)
                    pst = ps.tile([P, D], F32, tag="hT")
                    nc.tensor.transpose(pst[0:pj, :], h_s[:, j:j + pj],
                                        ident[0:D, 0:D])
                    hr = sb.tile([P, D], F32, tag="hr")
                    nc.scalar.activation(hr[0:pj, :], pst[0:pj, :], AF.Copy)
                    nc.sync.dma_start(h_bounce[co + j:co + j + pj, :],
                                      hr[0:pj, :])

            # initial h from input x
            with tc.tile_pool(name="h0", bufs=3) as hp, \
                 tc.tile_pool(name="h0p", bufs=2, space="PSUM") as hpp:
                for (co, cn) in ccols:
                    xc_s = hp.tile([D, CCH], F32, tag="xc")
                    nc.sync.dma_start(xc_s[:, 0:cn], xT_d[:, co:co + cn])
                    h_chunk(hp, hpp, xc_s, co, cn)

            col_off = 0
            for k in range(K):
                nc.gpsimd.collective_compute(
                    "AllGather", mybir.AluOpType.bypass, replica_groups=RG,
                    ins=[h_bounce.ap().opt()], outs=[h_full.ap().opt()])
                spmm(k, 0, h_full, col_off)
                nc.sync.dma_start(
                    edges_o[k, :, :].rearrange("(w p) d -> p w d", p=P),
                    acc[:].rearrange("p (w d) -> p w d", d=D))
                nc.sync.dma_start(
                    e_bounce[:, :].rearrange("(w p) d -> p w d", p=P),
                    acc[:].rearrange("p (w d) -> p w d", d=D))
                nc.gpsimd.collective_compute(
                    "AllGather", mybir.AluOpType.bypass, replica_groups=RG,
                    ins=[e_bounce.ap().opt()], outs=[e_full.ap().opt()])
                col_off = spmm(k, 1, e_full, col_off)

                with tc.tile_pool(name=f"po{k}", bufs=3) as po, \
                     tc.tile_pool(name=f"poP{k}", bufs=2, space="PSUM") as poP:
                    nbv = 14
                    for w0 in range(0, NT, nbv):
                        bw = min(nbv, NT - w0)
                        sl = acc[:, w0 * D:(w0 + bw) * D]
                        sl3 = sl.rearrange("p (b d) -> p b d", d=D)
                        nc.scalar.activation(sl, sl, AF.Exp)
                        ssum = po.tile([P, nbv], F32, tag="ssum")
                        nc.vector.reduce_sum(ssum[:, 0:bw], sl3,
                                             axis=mybir.AxisListType.X)
                        nc.vector.reciprocal(ssum[:, 0:bw], ssum[:, 0:bw])
                        nc.vector.tensor_tensor(
                            out=sl3, in0=sl3,
                            in1=ssum[:, 0:bw].unsqueeze(2).to_broadcast(
                                [P, bw, D]),
                            op=mybir.AluOpType.mult)
                    for (co, cn) in ccols:
                        psT = poP.tile([D, CCH], F32, tag="T")
                        for j in range(0, cn, P):
                            pj = min(P, cn - j)
                            w = (co + j) // P
                            nc.tensor.transpose(
                                psT[:, j:j + pj],
                                acc[:, w * D:(w + 1) * D][0:pj, :],
                                ident[0:pj, 0:pj])
                        smT = po.tile([D, CCH], F32, tag="smT")
                        nc.scalar.activation(smT[:, 0:cn], psT[:, 0:cn],
                                             AF.Copy)
                        psN = poP.tile([D, CCH], F32, tag="N")
                        nc.tensor.matmul(psN[:, 0:cn], lhsT=wfc1[:, :],
                                         rhs=smT[:, 0:cn], start=True,
                                         stop=True)
                        nodeT = po.tile([D, CCH], F32, tag="nodeT")
                        nc.scalar.activation(nodeT[:, 0:cn], psN[:, 0:cn],
                                             AF.Copy)
                        xc_s = po.tile([D, CCH], F32, tag="xc")
                        nc.sync.dma_start(xc_s[:, 0:cn], xT_d[:, co:co + cn])
                        psG = poP.tile([D, CCH], F32, tag="T")
                        nc.tensor.matmul(psG[:, 0:cn], lhsT=wfus[:, :],
                                         rhs=xc_s[:, 0:cn], start=True,
                                         stop=True)
                        t1x = po.tile([D, CCH], F32, tag="t1x")
                        nc.scalar.activation(t1x[:, 0:cn], psG[:, 0:cn],
                                             AF.Tanh, bias=b1c[:, 0:1])
                        psA0 = poP.tile([1, CCH], F32, tag="A")
                        nc.tensor.matmul(psA0[:, 0:cn], lhsT=w2c[:, :],
                                         rhs=t1x[:, 0:cn], start=True,
                                         stop=True)
                        psG2 = poP.tile([D, CCH], F32, tag="N")
                        nc.tensor.matmul(psG2[:, 0:cn], lhsT=wfus[:, :],
                                         rhs=nodeT[:, 0:cn], start=True,
                                         stop=True)
                        t1n = po.tile([D, CCH], F32, tag="t1n")
                        nc.scalar.activation(t1n[:, 0:cn], psG2[:, 0:cn],
                                             AF.Tanh, bias=b1c[:, 0:1])
                        psA1 = poP.tile([1, CCH], F32, tag="A")
                        nc.tensor.matmul(psA1[:, 0:cn], lhsT=w2c[:, :],
                                         rhs=t1n[:, 0:cn], start=True,
                                         stop=True)
                        a1s = po.tile([1, CCH], F32, tag="a1s")
                        nc.scalar.activation(a1s[:, 0:cn], psA1[:, 0:cn],
                                             AF.Copy)
                        s0 = po.tile([1, CCH], F32, tag="s0")
                        nc.vector.tensor_tensor(out=s0[:, 0:cn],
                                                in0=psA0[:, 0:cn],
                                                in1=a1s[:, 0:cn],
                                                op=mybir.AluOpType.subtract)
                        nc.scalar.activation(s0[:, 0:cn], s0[:, 0:cn],
                                             AF.Sigmoid)
                        s0b = poP.tile([D, CCH], F32, tag="A")
                        nc.tensor.matmul(s0b[:, 0:cn], lhsT=ones1[:, :],
                                         rhs=s0[:, 0:cn], start=True,
                                         stop=True)
                        diff = po.tile([D, CCH], F32, tag="diff")
                        nc.vector.tensor_tensor(out=diff[:, 0:cn],
                                                in0=xc_s[:, 0:cn],
                                                in1=nodeT[:, 0:cn],
                                                op=mybir.AluOpType.subtract)
                        nc.vector.tensor_tensor(
                            out=diff[:, 0:cn], in0=diff[:, 0:cn],
                            in1=s0b[:, 0:cn],
                            op=mybir.AluOpType.mult)
                        xnew = po.tile([D, CCH], F32, tag="xnew")
                        nc.vector.tensor_tensor(out=xnew[:, 0:cn],
                                                in0=nodeT[:, 0:cn],
                                                in1=diff[:, 0:cn],
                                                op=mybir.AluOpType.add)
                        nc.sync.dma_start(xT_d[:, co:co + cn], xnew[:, 0:cn])
                        nc.sync.dma_start(nodesT_o[k, :, co:co + cn],
                                          xnew[:, 0:cn])
                        if k < K - 1:
                            h_chunk(po, poP, xnew, co, cn)
    nc.compile()
    return nc


_CACHE = {}


def _get_nc(cfg, plan):
    key = (cfg.NU, cfg.NI, cfg.K, cfg.E, cfg.CH,
           tuple(tuple(map(tuple, nt)) for nt in plan["nt"]))
    if key not in _CACHE:
        _CACHE[key] = build(cfg, plan)
    return _CACHE[key]


def prepare(cfg, x, hgc1_bias, fc1_W, fus_l1_W, fus_l1_b, fus_l2_W, fus_l2_b,
            rows, cols):
    """Host prep: build (cached) module + per-core input maps."""
    x = np.asarray(x, np.float32)
    plan, per_core = prep(cfg, rows, cols)
    nc = _get_nc(cfg, plan)

    core_of, loc_of, _ = _perm_maps(cfg)
    in_maps = []
    for m in range(NCORES):
        xm = np.zeros((cfg.SHP, D), np.float32)
        sel = core_of == m
        xm[loc_of[sel]] = x[sel]
        in_maps.append({
            "xT": np.ascontiguousarray(xm.T),
            "biascol": np.asarray(hgc1_bias, np.float32).reshape(D, 1),
            "fc1_WT": np.ascontiguousarray(np.asarray(fc1_W, np.float32).T),
            "fus1_WT": np.ascontiguousarray(np.asarray(fus_l1_W, np.float32).T),
            "b1col": np.asarray(fus_l1_b, np.float32).reshape(D, 1),
            "w2col": np.ascontiguousarray(
                np.asarray(fus_l2_W, np.float32).reshape(1, D).T),
            "gidx": per_core[m]["gidx"],
            "val": per_core[m]["val"],
            "srel": per_core[m]["srel"],
        })
    return nc, in_maps


def unshard(cfg, results):
    """Per-core outputs -> full (nodes, edges)."""
    core_of, loc_of, _ = _perm_maps(cfg)
    N = cfg.NU + cfg.NI
    nodes = np.zeros((cfg.K, N, D), np.float32)
    edges = np.zeros((cfg.K, N, D), np.float32)
    for m in range(NCORES):
        sel = core_of == m
        nodesT = np.asarray(results[m]["nodesT"]).reshape(cfg.K, D, cfg.SHP)
        edg = np.asarray(results[m]["edges"]).reshape(cfg.K, cfg.SHP, D)
        nodes[:, sel, :] = nodesT[:, :, loc_of[sel]].transpose(0, 2, 1)
        edges[:, sel, :] = edg[:, loc_of[sel], :]
    return nodes, edges


def run(cfg, x, hgc1_bias, fc1_W, fus_l1_W, fus_l1_b, fus_l2_W, fus_l2_b,
        rows, cols, sim=False):
    nc, in_maps = prepare(cfg, x, hgc1_bias, fc1_W, fus_l1_W, fus_l1_b,
                          fus_l2_W, fus_l2_b, rows, cols)

    if sim:
        from concourse import bass_interp
        simu = bass_interp.MultiCoreSim(nc, NCORES)
        for m in range(NCORES):
            for kk, a in in_maps[m].items():
                simu.cores[m].tensor(kk)[:] = a
        simu.simulate()
        results = [{"nodesT": simu.cores[m].mem_tensor("nodesT"),
                    "edges": simu.cores[m].mem_tensor("edges")}
                   for m in range(NCORES)]
        exec_ns = None
    else:
        from concourse.bass_utils import run_bass_kernel_spmd
        import os
        trace = bool(int(os.environ.get("KERNEL_TRACE", "0")))
        res = run_bass_kernel_spmd(
            nc, in_maps, core_ids=list(range(NCORES)),
            trace=trace, trace_cores=[0] if trace else None,
            tmpdir="/tmp/hgnn_trace" if trace else None)
        results = res.results
        exec_ns = res.exec_time_ns

    return unshard(cfg, results), exec_ns


def kernel(x, hgc1_bias, fc1_W, fus_l1_W, fus_l1_b, fus_l2_W, fus_l2_b,
           rows, cols):
    cfg = Cfg(NU=50000, NI=50000, K=4, E=1000000, CH=8192)
    (nodes, edges), _ = run(cfg, x, hgc1_bias, fc1_W, fus_l1_W, fus_l1_b,
                            fus_l2_W, fus_l2_b, rows, cols)
    return nodes, edges

